# revision 35
# baseline (speedup 1.0000x reference)
"""BQuantConv1d Trainium2 kernel.

Math: reference's 256-entry LUT gather is algebraically a sum of 8 sign-matmuls.
For byte c = binary[0,k,m,f], table[b,m,c] = sum_p sgn(bit_{7-p}(c)) * x[b, m*8+p],
so out[b,f] = sum_k scale[k,f] * (xf @ Wk)[b,f] + bias[f] with
Wk[m*8+p, f] = 2*bit_{7-p}(binary[0,k,m,f]) - 1.

Using sum((2*bit-1)*x) = 2*sum(bit*x) - sum(x), the device only needs {0, c_p}
weights (c_p any power of two, folded into an exact host pre-scale of x).
Decode is ONE int16 bitwise_and per bit plane, with NO int->float casts:
the host re-encodes each byte's bits into bf16 EXPONENT bit positions
(bits 11..14 of a uint16), so  enc & (1 << POS)  is already a valid bf16
{+0, 2^(2^(POS-7)-127)} when bitcast to bf16. Two encoded tensors carry 4
planes each (positions 11..14; lower positions would overflow the
compensating x scale). The -sum(x) correction term S[b] is a host row-sum
fed as input and fused into the PSUM->SBUF copy as a tensor_scalar subtract.

Sharding: one bit-plane k per NeuronCore (8 cores = 8 bits). Every core runs
the same program on its own binary[0,k] slice; the host combines the 8 raw
partials with scale and bias.
"""

import numpy as np
import ml_dtypes

B = 256          # flattened tokens 4*64
NX = 768         # input features
NF = 768         # output features
M = 96           # groups of 8 input features
BITS = 8
NT = 384         # matmul free-dim tile (2 tiles of 384 = 768)

_CACHE = {}


def _emit_body(nc, tc, bass, mybir, pools, xt_d, bin_d, s_d, out_d, mode="full"):
    fp32 = mybir.dt.float32
    bf16 = mybir.dt.bfloat16
    const, wpool, opool, psum = pools
    do_decode = mode in ("full", "nomm")
    do_mm = mode == "full"
    if True:
        if True:
            # HWDGE descriptor generation serializes across dma_starts
            # (~0.6us each), so use few, large DMAs: enc first (heads the
            # decode chain), then xt, then the packed correction vector.
            binary = const.tile([M, 2 * NF], mybir.dt.int16, tag="bin")
            nc.sync.dma_start(binary[:], bin_d.ap())
            xt = const.tile([M, BITS * B], bf16, tag="xt")
            H = 4 * B  # first four p-blocks
            nc.scalar.dma_start(xt[:, 0:H], xt_d.ap()[:, 0:H])
            nc.scalar.dma_start(xt[:, H : 2 * H], xt_d.ap()[:, H : 2 * H])
            s_sb = const.tile([128, 2], fp32, tag="s")
            nc.sync.dma_start(s_sb[:], s_d.ap())

            if not do_decode:
                return

            pm = [
                [
                    psum.tile([128, NT], fp32, tag=f"pm{m}{n}", name=f"pm{m}{n}")
                    for n in range(2)
                ]
                for m in range(2)
            ]

            for p in range(8):
                half = p // 4  # which encoded tensor carries this plane
                pos = 14 - (p % 4)  # exponent bit position
                wi = wpool.tile([M, NF], mybir.dt.int16, tag="wi")
                nc.vector.tensor_scalar(
                    wi[:],
                    binary[:, half * NF : (half + 1) * NF],
                    1 << pos, None, mybir.AluOpType.bitwise_and,
                )
                w = wi.bitcast(bf16)  # {+0, 2^(2^(pos-7)-127)} exactly
                if not do_mm:
                    continue
                for m in range(2):
                    lhsT = xt[:, p * B + m * 128 : p * B + (m + 1) * 128]
                    for n in range(2):
                        nc.tensor.matmul(
                            pm[m][n][:, :], lhsT, w[:, n * NT : (n + 1) * NT],
                            start=(p == 0), stop=(p == 7),
                        )

            if not do_mm:
                return
            for m in range(2):
                out_sb = opool.tile([128, NF], fp32, tag="out")
                for n in range(2):
                    nc.vector.tensor_scalar(
                        out_sb[:, n * NT : (n + 1) * NT], pm[m][n][:, :],
                        s_sb[:, m : m + 1], None, mybir.AluOpType.subtract,
                    )
                eng = nc.sync if m == 0 else nc.scalar
                eng.dma_start(out_d.ap()[m * 128 : (m + 1) * 128, :], out_sb[:])


def _declare_io(nc, mybir):
    fp32 = mybir.dt.float32
    bf16 = mybir.dt.bfloat16
    # xt[m, p*256 + b] = xf[b, m*8+p] * 2^(128 - 2^(POS_p - 7)), bf16
    xt_d = nc.dram_tensor("xt", [M, BITS * B], bf16, kind="ExternalInput")
    # exponent-encoded bit planes, two uint16 halves side by side
    bin_d = nc.dram_tensor("bin", [M, 2 * NF], mybir.dt.int16, kind="ExternalInput")
    # S[r, m] = sum_n xf[m*128 + r, n] (host row-sum, packed for one DMA)
    s_d = nc.dram_tensor("s", [128, 2], fp32, kind="ExternalInput")
    # raw partial: out_k = xf @ Wk  (f32)
    out_d = nc.dram_tensor("out", [B, NF], fp32, kind="ExternalOutput")
    return xt_d, bin_d, s_d, out_d


def _build_program(n_iter=1, mode="full"):
    import concourse.bass as bass
    import concourse.tile as tile
    from concourse import bacc, mybir

    nc = bacc.Bacc("TRN2", target_bir_lowering=False, debug=False)
    io = _declare_io(nc, mybir)

    with tile.TileContext(nc) as tc:
        with (
            tc.tile_pool(name="const", bufs=1) as const,
            tc.tile_pool(name="wpool", bufs=4) as wpool,
            tc.tile_pool(name="opool", bufs=2) as opool,
            tc.tile_pool(name="psum", bufs=1, space=bass.MemorySpace.PSUM) as psum,
        ):
            pools = (const, wpool, opool, psum)
            if n_iter == 1:
                _emit_body(nc, tc, bass, mybir, pools, *io, mode=mode)
            else:
                with tc.For_i(0, n_iter, 1):
                    if mode == "empty":
                        zz = const.tile([128, 1], mybir.dt.float32, tag="zz")
                        nc.gpsimd.memset(zz[:], 0.0)
                    else:
                        _emit_body(nc, tc, bass, mybir, pools, *io, mode=mode)

    nc.compile()
    return nc


def _prep_inputs(x, binary):
    xf = np.asarray(x, dtype=np.float32).reshape(B, NX)
    # xt3[m, p, b] = xf[b, m*8+p] * 2^(128 - exp_p), where plane p's decoded
    # weight is 2^(exp_p - 127) with exp_p = 2^(POS_p - 7), POS_p = 14 - p%4.
    # Product = 2 * bit * x exactly (pure exponent arithmetic).
    xt3 = xf.reshape(B, M, 8).transpose(1, 2, 0).astype(np.float32)
    exp_p = np.array([1 << (14 - (p % 4) - 7) for p in range(8)])  # 128,64,32,16,...
    pw = (2.0 ** (128 - exp_p)).astype(np.float32)
    xt3 = xt3 * pw[None, :, None]
    xt = np.ascontiguousarray(xt3).reshape(M, BITS * B).astype(ml_dtypes.bfloat16)

    # correction term: the kernel computes 2*sum(bit*x) and subtracts S.
    # Use bf16(x) sums so the correction matches the matmul's operand rounding.
    # Packed [128, 2]: column m holds S for token block m*128..(m+1)*128.
    s = xf.astype(ml_dtypes.bfloat16).astype(np.float32).sum(axis=1)
    s = np.ascontiguousarray(s.reshape(2, 128).T).astype(np.float32)

    # exponent-encode: half h carries planes p = 4h..4h+3 of byte bit q = 7-p
    # at uint16 position POS = 14 - (p % 4).
    bins = np.asarray(binary, dtype=np.int32)[0]  # [8, 96, 768] values 0..255
    bit = (bins[:, :, :, None] >> (7 - np.arange(8))[None, None, None, :]) & 1
    # bit[k, m, f, p] in {0, 1}
    pos = 14 - (np.arange(4))  # POS for p%4 = 0..3
    enc = np.zeros((BITS, M, 2, NF), dtype=np.int32)
    for p in range(8):
        enc[:, :, p // 4, :] |= bit[:, :, :, p] << (14 - (p % 4))
    enc = enc.reshape(BITS, M, 2 * NF).astype(np.uint16).view(np.int16)

    in_maps = [
        {"xt": xt, "bin": np.ascontiguousarray(enc[k]), "s": s}
        for k in range(BITS)
    ]
    return in_maps


def kernel(x, scale, binary, bias, _trace=False):
    from concourse.bass_utils import run_bass_kernel_spmd

    if "nc" not in _CACHE:
        _CACHE["nc"] = _build_program()
    nc = _CACHE["nc"]

    in_maps = _prep_inputs(x, binary)
    res = run_bass_kernel_spmd(nc, in_maps, core_ids=list(range(BITS)), trace=_trace)
    _CACHE["last_result"] = res

    outs = np.stack([res.results[k]["out"] for k in range(BITS)])  # [8, 256, 768]
    scale_np = np.asarray(scale, dtype=np.float32)[0]  # [8, 768]
    final = (outs * scale_np[:, None, :]).sum(axis=0) + np.asarray(
        bias, dtype=np.float32
    )[None, :]
    return final.reshape(4, 64, NF).astype(np.float32)
